# revision 5
# baseline (speedup 1.0000x reference)
"""AdMSoftmaxLoss distributed Trainium2 kernel (v4).

Reference computation (N=8192, D=1024, C=10240, S=30, ml=0.4, ms=0.1):
    wf    = clip(l2norm(x) @ l2norm(weight).T, -1, 1)      # (N, C) cosines
    m     = where(labels <= 5, ml, ms)
    t     = wf[i, labels[i]]
    num   = S * (t - m)
    excl  = sum_j exp(S * wf[i, j]) - exp(S * t)
    L     = num - log(exp(num) + excl)
    loss  = -mean(L)

Sharding: 2 row-groups x 4 class-groups over 8 NeuronCores. Core i gets
rows [(i//4)*4096, ..) and classes [(i%4)*2560, ..).

Work split:
  - Host (O((N+C)*D), ~0.01% of total FLOPs): l2-normalize x and weight,
    scale by 64, quantize to fp8e4m3 (clipped to +-240 so the OCP grid
    matches TRN fp8e4), transpose to d-major, and gather the label target
    t[i] = xn[i] . wn[labels[i]] with one einsum.
  - Device (Theta(N*C*D) matmul + Theta(N*C) exp/row-sum): per core,
    cos*4096 = x8^T . w8 via fp8 DoubleRow matmuls (K=256 per
    instruction), ScalarE exp(psum * 30/4096) -> bf16, VectorE row-sum.
    Output: out[p, m] = sum_{c in shard} exp(S*cos[row, c]), row=m*128+p.
  - Host epilogue: total = sum of 4 class-shard partials, then the O(N)
    loss arithmetic in f64.

Schedule notes (from perfetto traces):
  - ~7us fixed NEFF preamble before the first user instruction; first
    DMA data lands ~9us. Dummy warm-up matmuls fill that window so the
    PE HAM clock-gate is at 8/8 when the real stream starts.
  - All 5 w chunks go on the sync ring (starts earliest); x chunks on
    scalar/gpsimd rings. For the first x-group the class-chunk loop is
    outermost so matmuls only need w_n at ~n*3.4us.
  - PSUM: 6 x 1-bank [128, 512] tiles; each (m-tile, chunk) pair is 4
    DoubleRow matmuls (kp accumulation) -> ScalarE exp -> DVE reduce.
"""

import os
import sys
import types

import numpy as np


def _ensure_ntff_hook():
    """Make bass_utils' trace=True path usable: some containers ship an
    antenv stub without axon_hooks, which crashes run_bass_kernel_spmd
    when tracing is requested. Install the ctypes-based hook shim."""
    try:
        import antenv.axon_hooks  # noqa: F401

        return
    except ImportError:
        pass
    try:
        from trn_agent_boot.trn_boot import _ntff_profile_via_ctypes

        hook = _ntff_profile_via_ctypes("/opt/axon/libaxon_pjrt.so")
    except Exception:
        hook = None
    mod = types.ModuleType("antenv.axon_hooks")
    mod.get_axon_ntff_profile_hook = lambda: hook
    mod.set_axon_ntff_profile_hook = lambda h: None
    sys.modules["antenv.axon_hooks"] = mod
    try:
        import antenv

        antenv.axon_hooks = mod
    except ImportError:
        pass


P = 128
N_ROWS, D, C = 8192, 1024, 10240
S = 30.0
ML, MS = 0.4, 0.1
NCORES = 8
RG, CG = 2, 4                  # row groups x class groups
R_LOC = N_ROWS // RG           # 4096
C_LOC = C // CG                # 2560
M_TILES = R_LOC // P           # 32
K_TILES = D // P               # 8
NCHUNK = 512
N_CHUNKS = C_LOC // NCHUNK     # 5
X_CHUNKS = R_LOC // NCHUNK     # 8
JPC = NCHUNK // P              # 4 m-tiles per x chunk
N_WARM = 56                    # PE warm-up matmuls (~6us at cold clock)

QSCALE = 64.0                  # fp8 quantization scale for xn and wn
SEXP = S / (QSCALE * QSCALE)   # exp scale: psum = QSCALE^2 * cos

_CACHE = {}
LAST_RESULTS = None  # BassKernelResults of the most recent run (for test.py)


def _build():
    """Build + compile the SPMD Bass graph once; cache in module global."""
    if "nc" in _CACHE:
        return _CACHE["nc"]

    import concourse.bass as bass
    import concourse.mybir as mybir
    import concourse.tile as tile
    from concourse import bacc

    dt = mybir.dt
    AF = mybir.ActivationFunctionType
    ALU = mybir.AluOpType

    nc = bacc.Bacc(
        "TRN2", target_bir_lowering=False, debug=False, num_devices=NCORES
    )

    x_ext = nc.dram_tensor(
        "x8", [P, K_TILES, R_LOC], dt.float8e4, kind="ExternalInput"
    ).ap()
    w_ext = nc.dram_tensor(
        "w8", [P, K_TILES, C_LOC], dt.float8e4, kind="ExternalInput"
    ).ap()
    out_ext = nc.dram_tensor(
        "out", [P, M_TILES], dt.float32, kind="ExternalOutput"
    ).ap()

    with tile.TileContext(nc) as tc:
        with (
            tc.tile_pool(name="consts", bufs=1) as consts,
            tc.tile_pool(name="esc", bufs=3) as esc,
            tc.tile_pool(name="psum", bufs=6, space="PSUM") as psum,
            tc.tile_pool(name="wps", bufs=1, space="PSUM") as wps,
        ):
            wsb = [
                consts.tile([P, K_TILES, NCHUNK], dt.float8e4, name=f"w{n}")
                for n in range(N_CHUNKS)
            ]
            xsb = [
                consts.tile([P, K_TILES, NCHUNK], dt.float8e4, name=f"x{g}")
                for g in range(X_CHUNKS)
            ]
            sums = consts.tile([P, M_TILES, N_CHUNKS], dt.float32)
            outsum = consts.tile([P, M_TILES], dt.float32)
            warm = consts.tile([P, P], dt.bfloat16)

            # Warm-up source + input DMAs. gpsimd memsets `warm` first so
            # the PE can run dummy matmuls while real data streams in.
            nc.gpsimd.memset(warm[:], 0.0)
            for n in range(N_CHUNKS):
                nc.sync.dma_start(wsb[n][:], w_ext[:, :, bass.ts(n, NCHUNK)])
            for g in range(2):
                nc.scalar.dma_start(xsb[g][:], x_ext[:, :, bass.ts(g, NCHUNK)])
            for g in range(2, X_CHUNKS):
                nc.gpsimd.dma_start(xsb[g][:], x_ext[:, :, bass.ts(g, NCHUNK)])

            # PE HAM warm-up: ~6us of back-to-back tiny matmuls so the
            # clock gate reaches 8/8 before the first real matmul.
            wp = wps.tile([P, P], dt.float32)
            for _ in range(N_WARM):
                nc.tensor.matmul(wp[:], warm[:], warm[:], start=True, stop=True)

            def do_tile(g, n, j):
                """m-tile j of x-chunk g vs class-chunk n."""
                jg = g * JPC + j
                ps = psum.tile([P, NCHUNK], dt.float32, tag="ps")
                for kp in range(K_TILES // 2):
                    nc.tensor.matmul(
                        ps[:],
                        xsb[g][:, 2 * kp : 2 * kp + 2, bass.ts(j, P)],
                        wsb[n][:, 2 * kp : 2 * kp + 2, :],
                        start=(kp == 0),
                        stop=(kp == K_TILES // 2 - 1),
                        perf_mode=mybir.MatmulPerfMode.DoubleRow,
                    )
                e = esc.tile([P, NCHUNK], dt.bfloat16, tag="esc")
                nc.scalar.activation(e[:], ps[:], AF.Exp, scale=SEXP)
                nc.vector.tensor_reduce(
                    sums[:, jg, n : n + 1],
                    e[:],
                    axis=mybir.AxisListType.X,
                    op=ALU.add,
                )

            for g in range(X_CHUNKS):
                if g == 0:
                    # chunk-outer: w_n first needed at ~n*3.4us
                    for n in range(N_CHUNKS):
                        for j in range(JPC):
                            do_tile(g, n, j)
                else:
                    for j in range(JPC):
                        for n in range(N_CHUNKS):
                            do_tile(g, n, j)

            nc.vector.tensor_reduce(
                outsum[:], sums[:], axis=mybir.AxisListType.X, op=ALU.add
            )
            nc.sync.dma_start(out_ext, outsum[:])

    nc.compile()
    _CACHE["nc"] = nc
    return nc


def _quant8(a):
    """f32 -> TRN fp8e4 grid (OCP e4m3fn clipped to +-240)."""
    import ml_dtypes

    return np.clip(a, -240.0, 240.0).astype(ml_dtypes.float8_e4m3fn)


def _make_in_maps(xn8, wn8):
    """Shard + transpose to [P, K_TILES, cols] d-major layouts."""
    in_maps = []
    for i in range(NCORES):
        gr, ci = divmod(i, CG)
        xs = xn8[gr * R_LOC : (gr + 1) * R_LOC]  # (R_LOC, D)
        ws = wn8[ci * C_LOC : (ci + 1) * C_LOC]  # (C_LOC, D)
        # [r, k*128+p] -> [p, k, r]
        xT = np.ascontiguousarray(
            xs.T.reshape(K_TILES, P, R_LOC).transpose(1, 0, 2)
        )
        wT = np.ascontiguousarray(
            ws.T.reshape(K_TILES, P, C_LOC).transpose(1, 0, 2)
        )
        in_maps.append({"x8": xT, "w8": wT})
    return in_maps


def kernel(x, labels, weight):
    global LAST_RESULTS
    from concourse.bass_utils import run_bass_kernel_spmd

    x = np.asarray(x, dtype=np.float32)
    weight = np.asarray(weight, dtype=np.float32)
    labels = np.asarray(labels)

    # Host: normalize (eps matches F.normalize), quantize, target gather.
    xn = x / np.maximum(np.linalg.norm(x, axis=1, keepdims=True), 1e-12)
    wn = weight / np.maximum(
        np.linalg.norm(weight, axis=1, keepdims=True), 1e-12
    )
    t = np.clip(np.einsum("nd,nd->n", xn, wn[labels]), -1.0, 1.0)
    xn8 = _quant8(QSCALE * xn)
    wn8 = _quant8(QSCALE * wn)

    nc = _build()
    in_maps = _make_in_maps(xn8, wn8)
    trace = bool(int(os.environ.get("ADMS_TRACE", "0")))
    res = run_bass_kernel_spmd(nc, in_maps, list(range(NCORES)), trace=trace)
    LAST_RESULTS = res

    total = np.zeros(N_ROWS, np.float64)
    for i, r in enumerate(res.results):
        gr = i // CG
        o = np.asarray(r["out"], dtype=np.float64)  # [P, M_TILES]
        sl = slice(gr * R_LOC, (gr + 1) * R_LOC)
        total[sl] += o.T.reshape(R_LOC)  # row = m*P + p

    t = t.astype(np.float64)
    m = np.where(labels <= 5, ML, MS)
    num = S * (t - m)
    L = num - np.log(np.exp(num) + (total - np.exp(S * t)))
    return np.float32(-L.mean())
